# revision 20
# baseline (speedup 1.0000x reference)
"""Gated MSA-row attention (AlphaFold-style) Trainium2 kernel.

Sharding: data-parallel over the 128 MSA rows -> 16 rows/core on 8 cores;
rows processed in pairs, fully pipelined (no separate phases).

- transposed activations (qT/kT [hc, (row,pos)]) -> no on-chip transposes
- logitsT [k, q]: denominators via a weighted-ones matmul over partitions
  (weights = 0.5*exp(b1-2) -> bias1, the softmax shift and the sigmoid
  1/2 all ride along for free), emerging pre-broadcast in per-head
  32-strips; reciprocal_approx_fast
- exp on ACT straight from PSUM, batched [128,2048], constant bias -4
- gate uses tanh (same ACT table set as exp -> zero table switches):
  sigmoid(x) = (1 + tanh(x/2))/2, the (1+t) via scalar_tensor_tensor in
  the gating multiply, the 1/2 inside the denominator weights
- pair bias: a' = E * exp(bias2) fp16 2x TT, emitted per (row, head
  group) right after each exp so row 0's a' slices are ready two exps
  earlier and the denominator/AV back-phase starts sooner; bias1 also
  scales v (DVE tensor_scalar on the v evacuation)
- all matmul operands fp16 (fp32 PSUM accumulation)
- every 4-pack of concurrent row-tiled matmuls uses 4 distinct PSUM banks
  (same-bank concurrent drains at full rate = HW write-port collision)
- fp16 output evacuation + fp16 DMA out (host upcasts to fp32): halves
  the store traffic, error stays ~9e-4 vs the 2e-2 gate
- kv/q activation DMAs for pair p+1 issue during pair p's front phase
  (double-buffered pools), hiding the load latency entirely
"""

import math
import sys

sys.path.insert(0, "/opt/trn_rl_repo")

import numpy as np

import concourse.bass as bass
import concourse.mybir as mybir
from concourse import bacc
from concourse.tile import TileContext

F32 = mybir.dt.float32
F16 = mybir.dt.float16

H = 8
CH = 32
CQ = 256
Q = 256
K = 256
S = 128
NCORES = 8
RPC = S // NCORES
PAIRS = RPC // 2

Exp = mybir.ActivationFunctionType.Exp
Tanh = mybir.ActivationFunctionType.Tanh
MULT = mybir.AluOpType.mult
ADD = mybir.AluOpType.add


def _emit_dma(nc, P, p):
    kv = [P["kvx"].tile([128, 512], F16, tag=f"kv{c}", name=f"kv{c}")
          for c in range(2)]
    qx = [P["qxp"].tile([128, 512], F16, tag=f"qx{c}", name=f"qx{c}")
          for c in range(2)]
    for c in range(2):
        nc.gpsimd.dma_start(kv[c][:], P["kvx_d"][p, c * 128:(c + 1) * 128, :])
        nc.gpsimd.dma_start(qx[c][:], P["qx_d"][p, c * 128:(c + 1) * 128, :])
    return kv, qx


def _emit_front(nc, tc, P, p, kvqx):
    row = [p * 2, p * 2 + 1]
    ps_lt, ps_kv = P["pslt"], P["pskv"]
    kv, qx = kvqx

    # kT / qT projections -> fp16 [hc-chunk, (r,pos)]
    kt_sb, qt_sb, gt_sb = [], [], []
    for m in range(2):
        msl = slice(m * 128, (m + 1) * 128)
        with nc.named_scope("ktproj"):
            pk = ps_kv.tile([128, 512], F32, tag="kv", name="pkv")
            nc.tensor.matmul(pk[:], P["wk_sb"][0][:, msl], kv[0][:],
                             start=True, stop=False)
            nc.tensor.matmul(pk[:], P["wk_sb"][1][:, msl], kv[1][:],
                             start=False, stop=True)
            kt = P["kt"].tile([128, 512], F16, tag=f"kt{m}", name=f"kt{m}")
            nc.vector.tensor_copy(kt[:], pk[:])
            kt_sb.append(kt)
        with nc.named_scope("qproj"):
            pq = ps_kv.tile([128, 512], F32, tag="kv", name="pkv")
            nc.tensor.matmul(pq[:], P["wq_sb"][0][:, msl], qx[0][:],
                             start=True, stop=False)
            nc.tensor.matmul(pq[:], P["wq_sb"][1][:, msl], qx[1][:],
                             start=False, stop=True)
            qt = P["qt"].tile([128, 512], F16, tag=f"qt{m}", name=f"qt{m}")
            nc.vector.tensor_copy(qt[:], pq[:])
            qt_sb.append(qt)

    # v projection -> fp16 [k-chunk, (kc,hc)] per row, scaled by eb1
    v_sb = []
    for r in range(2):
        with nc.named_scope("vproj"):
            pv = ps_kv.tile([128, 512], F32, tag="kv", name="pkv")
            for kc in range(2):
                for c in range(2):
                    nc.tensor.matmul(
                        pv[:, kc * 256:(kc + 1) * 256],
                        kv[c][:, r * 256 + kc * 128:r * 256 + kc * 128 + 128],
                        P["wv_sb"][c][:],
                        start=(c == 0), stop=(c == 1))
            vt = P["vt"].tile([128, 512], F16, tag=f"v{r}", name=f"v{r}")
            for kc in range(2):
                eng = nc.scalar.mul if r else nc.vector.tensor_scalar_mul
                eng(vt[:, kc * 256:(kc + 1) * 256],
                    pv[:, kc * 256:(kc + 1) * 256],
                    P["eb1_sb"][:, row[r] * 2 + kc:row[r] * 2 + kc + 1])
            v_sb.append(vt)

    # gate projection -> tanh(x/2) on ACT (same table set as exp)
    for m in range(2):
        msl = slice(m * 128, (m + 1) * 128)
        with nc.named_scope("gproj"):
            pg = ps_kv.tile([128, 512], F32, tag="kv", name="pkv")
            nc.tensor.matmul(pg[:], P["wg_sb"][0][:, msl], qx[0][:],
                             start=True, stop=False)
            nc.tensor.matmul(pg[:], P["wg_sb"][1][:, msl], qx[1][:],
                             start=False, stop=True)
            gt = P["gt"].tile([128, 512], F32, tag=f"gt{m}", name=f"gt{m}")
            nc.scalar.activation(gt[:], pg[:], Tanh, scale=0.5)
            gt_sb.append(gt)

    # logits (fp16) + exp into E [128, 8192]: col = kc*4096+blk*1024+hp*256+q
    e_sb = P["esb"].tile([128, 8192], F16, tag="e", name="e")
    ev = e_sb[:].rearrange("p (kc blk rest) -> p kc blk rest", kc=2, blk=4)
    a_sb = [P["asb"].tile([128, 4096], F16, tag=f"a{kc}", name=f"a{kc}")
            for kc in range(2)]
    for r in range(2):
        for hg in range(2):
            with nc.named_scope("logits"):
                # head hp owns PSUM bank hp of lt; kc0/kc1 in its halves
                lt = ps_lt.tile([128, 2048], F32, tag="lt", name="lt")
                for kc in range(2):
                    for hp in range(4):
                        h = hg * 4 + hp
                        m, st = h // 4, 32 * (h % 4)
                        nc.tensor.matmul(
                            lt[:, hp * 512 + kc * 256:
                               hp * 512 + kc * 256 + 256],
                            kt_sb[m][st:st + 32,
                                     r * 256 + kc * 128:
                                     r * 256 + kc * 128 + 128],
                            qt_sb[m][st:st + 32, r * 256:r * 256 + 256],
                            start=True, stop=True,
                            tile_position=(st, 0))
            with nc.named_scope("exp"):
                blk = hg * 2 + r
                eo = ev[:, :, blk, :].rearrange("p kc (hp q) -> p hp kc q",
                                                hp=4)
                nc.scalar.activation(eo, lt[:], Exp, bias=P["shift_sb"][:])
            for kc in range(2):
                with nc.named_scope("hadamard"):
                    blk = hg * 2 + r
                    hs = slice(hg * 2048 + r * 1024, hg * 2048 + r * 1024 + 1024)
                    nc.vector.tensor_tensor(
                        a_sb[kc][:, hg * 2048 + r * 1024:
                                 hg * 2048 + r * 1024 + 1024],
                        e_sb[:, kc * 4096 + blk * 1024:
                             kc * 4096 + blk * 1024 + 1024],
                        P["eb2_sb"][kc][:, hs], MULT)

    return {"row": row, "a_sb": a_sb, "v_sb": v_sb, "gt_sb": gt_sb}


def _emit_back(nc, tc, P, p, ctx):
    row, a_sb = ctx["row"], ctx["a_sb"]
    v_sb, gt_sb = ctx["v_sb"], ctx["gt_sb"]
    ps_ot, ps_misc = P["psot"], P["psmisc"]
    for r in range(2):
        # denominators: weighted ones-matmul, strided rhs covers both hgs
        bc = ps_misc.tile([128, 512], F32, tag="misc", name="bc")
        with nc.named_scope("denom"):
            for hp in range(4):
                for kc in range(2):
                    av = a_sb[kc][:].rearrange("p (hg b) -> p hg b", hg=2)
                    rhs = av[:, :, r * 1024 + hp * 256:
                             r * 1024 + hp * 256 + 256]
                    w1 = (row[r] * 2 + kc) * 32
                    nc.tensor.matmul(
                        bc[32 * hp:32 * hp + 32, :],
                        P["eb1w_sb"][:, w1:w1 + 32], rhs,
                        start=(kc == 0), stop=(kc == 1),
                        tile_position=(0, 32 * hp))
        rc = P["rcp"].tile([128, 512], F32, tag=f"rc{r}", name=f"rc{r}")
        with nc.named_scope("recip"):
            nc.vector.reciprocal_approx_fast(rc[:], bc[:])

        # AV
        ot = ps_ot.tile([128, 512], F32, tag="ot", name="ot")
        with nc.named_scope("av"):
            for hg in range(2):
                for hp in range(4):
                    for kc in range(2):
                        off = (hg * 2 + r) * 1024 + hp * 256
                        nc.tensor.matmul(
                            ot[32 * hp:32 * hp + 32,
                               hg * 256:hg * 256 + 256],
                            v_sb[r][:, kc * 256 + hg * 128 + 32 * hp:
                                    kc * 256 + hg * 128 + 32 * hp + 32],
                            a_sb[kc][:, off:off + 256],
                            start=(kc == 0), stop=(kc == 1),
                            tile_position=(0, 32 * hp))

        # gating: og = (1 + tanh) * oT * rc   (the 1/2s live in eb1w)
        og = []
        for hg in range(2):
            with nc.named_scope("gating"):
                csl = slice(hg * 256, hg * 256 + 256)
                tmp = P["osb"].tile([128, 256], F32, tag="gtmp", name="gtmp")
                nc.vector.scalar_tensor_tensor(
                    tmp[:], gt_sb[hg][:, r * 256:r * 256 + 256], 1.0,
                    ot[:, csl], ADD, MULT)
                ogt = P["otg"].tile([128, 256], F16, tag=f"og{hg}",
                                    name=f"og{hg}")
                nc.vector.tensor_tensor(ogt[:], tmp[:], rc[:, csl], MULT)
                og.append(ogt)

        fin = ps_misc.tile([128, 512], F32, tag="misc", name="misc")
        with nc.named_scope("outproj"):
            for qc in range(2):
                for hg in range(2):
                    nc.tensor.matmul(
                        fin[:, qc * 256:qc * 256 + 256],
                        og[hg][:, qc * 128:qc * 128 + 128],
                        P["wo_sb"][hg][:],
                        start=(hg == 0), stop=(hg == 1))
        with nc.named_scope("outevac"):
            if r == 0:
                ob = P["osb"].tile([128, 1024], F16, tag="ob", name="ob")
                ctx["ob"] = ob
            else:
                ob = ctx["ob"]
            nc.scalar.copy(ob[:, r * 512:(r + 1) * 512], fin[:])
        if r == 1:
            nc.sync.dma_start(
                P["out_d"][row[0]:row[0] + 2].rearrange(
                    "r (qc p) d -> p r qc d", qc=2),
                ob[:].rearrange("p (r qc d) -> p r qc d", r=2, qc=2))


def build_nc():
    nc = bacc.Bacc("TRN2", target_bir_lowering=False)

    P = {}
    P["qx_d"] = nc.dram_tensor("qx", [PAIRS, CQ, 512], F16,
                               kind="ExternalInput")
    P["kvx_d"] = nc.dram_tensor("kvx", [PAIRS, CQ, 512], F16,
                                kind="ExternalInput")
    wd = {nm: nc.dram_tensor(f"w{nm}t", [CQ, 256], F16, kind="ExternalInput")
          for nm in ("q", "k", "v", "g", "o")}
    b1_d = nc.dram_tensor("eb1s", [128, 2 * RPC], F32, kind="ExternalInput")
    b1w_d = nc.dram_tensor("eb1w", [128, 2 * RPC * 32], F16,
                           kind="ExternalInput")
    eb2_d = nc.dram_tensor("eb2", [K, 4096], F16, kind="ExternalInput")
    P["out_d"] = nc.dram_tensor("out", [RPC, Q, 256], F16,
                                kind="ExternalOutput")

    with TileContext(nc) as tc:
        with (
            tc.tile_pool(name="const", bufs=1) as cpool,
            tc.tile_pool(name="kvx", bufs=2) as kv_pool,
            tc.tile_pool(name="qxp", bufs=2) as qx_pool,
            tc.tile_pool(name="kt", bufs=2) as kt_pool,
            tc.tile_pool(name="qt", bufs=2) as qt_pool,
            tc.tile_pool(name="gt", bufs=2) as gt_pool,
            tc.tile_pool(name="vt", bufs=2) as vt_pool,
            tc.tile_pool(name="esb", bufs=2) as e_pool,
            tc.tile_pool(name="asb", bufs=2) as a_pool,
            tc.tile_pool(name="rcp", bufs=2) as r_pool,
            tc.tile_pool(name="otg", bufs=2) as og_pool,
            tc.tile_pool(name="osb", bufs=2) as o_pool,
            tc.tile_pool(name="pslt", bufs=1, space="PSUM") as ps_lt,
            tc.tile_pool(name="pskv", bufs=2, space="PSUM") as ps_kv,
            tc.tile_pool(name="psot", bufs=1, space="PSUM") as ps_ot,
            tc.tile_pool(name="psmisc", bufs=1, space="PSUM") as ps_misc,
        ):
            for nm in ("q", "k", "v", "g", "o"):
                tiles = [cpool.tile([128, 256], F16, tag=f"w{nm}{c}",
                                    name=f"w{nm}{c}") for c in range(2)]
                for c in range(2):
                    nc.sync.dma_start(tiles[c][:],
                                      wd[nm][c * 128:(c + 1) * 128, :])
                P[f"w{nm}_sb"] = tiles
            eb1_sb = cpool.tile([128, 2 * RPC], F32, tag="eb1", name="eb1")
            nc.sync.dma_start(eb1_sb[:], b1_d[:])
            P["eb1_sb"] = eb1_sb
            eb2_sb = [cpool.tile([128, 4096], F16, tag=f"eb2{c}",
                                 name=f"eb2{c}") for c in range(2)]
            for c in range(2):
                nc.sync.dma_start(eb2_sb[c][:], eb2_d[c * 128:(c + 1) * 128, :])
            P["eb2_sb"] = eb2_sb
            eb1w_sb = cpool.tile([128, 2 * RPC * 32], F16, tag="eb1w",
                                 name="eb1w")
            nc.sync.dma_start(eb1w_sb[:], b1w_d[:])
            P["eb1w_sb"] = eb1w_sb
            shift_sb = cpool.tile([128, 1], F32, tag="shift", name="shift")
            nc.vector.memset(shift_sb[:], -4.0)
            P["shift_sb"] = shift_sb

            P.update({"kvx": kv_pool, "qxp": qx_pool, "kt": kt_pool,
                      "qt": qt_pool, "gt": gt_pool, "vt": vt_pool,
                      "esb": e_pool, "asb": a_pool, "rcp": r_pool,
                      "otg": og_pool, "osb": o_pool, "pslt": ps_lt,
                      "pskv": ps_kv, "psot": ps_ot, "psmisc": ps_misc})
            loads = _emit_dma(nc, P, 0)
            for p in range(PAIRS):
                ctx = _emit_front(nc, tc, P, p, loads)
                if p + 1 < PAIRS:
                    loads = _emit_dma(nc, P, p + 1)
                _emit_back(nc, tc, P, p, ctx)

    nc.compile()
    return nc


def host_prep(q_x, kv_x, bias1, bias2, wq, wk, wv, wg, wo):
    wqt = np.ascontiguousarray((wq / math.sqrt(CH)).T.astype(np.float16))
    wkt = np.ascontiguousarray(wk.T.astype(np.float16))
    wvt = np.ascontiguousarray(wv.T.astype(np.float16))
    wgt = np.ascontiguousarray(wg.T.astype(np.float16))
    wot = np.ascontiguousarray(wo.T.astype(np.float16))

    b2 = bias2[0, 0]
    eb2 = np.exp(b2.astype(np.float32)).transpose(2, 0, 1)   # [K, H, Q]
    eb2 = eb2.reshape(K, 2, 4, Q)
    eb2 = np.broadcast_to(eb2[:, :, None, :, :], (K, 2, 2, 4, Q))
    eb2 = np.ascontiguousarray(eb2.reshape(K, 4096).astype(np.float16))

    in_maps = []
    for c in range(NCORES):
        rows = slice(c * RPC, (c + 1) * RPC)
        qx = q_x[0, rows]
        qxp = qx.reshape(PAIRS, 2, Q, CQ).transpose(0, 3, 1, 2)
        qxp = np.ascontiguousarray(qxp.reshape(PAIRS, CQ, 512)
                                   .astype(np.float16))
        kvx = kv_x[0, rows]
        kvp = kvx.reshape(PAIRS, 2, K, CQ).transpose(0, 3, 1, 2)
        kvp = np.ascontiguousarray(kvp.reshape(PAIRS, CQ, 512)
                                   .astype(np.float16))
        b1 = np.exp(bias1[0, rows, 0, 0, :].astype(np.float32) - 2.0)
        eb1s = np.ascontiguousarray(
            b1.reshape(RPC, 2, 128).transpose(2, 0, 1).reshape(128, 2 * RPC))
        # denominator weights also carry the sigmoid 1/2 (gating computes
        # (1+tanh) * oT * rc = 2*sigmoid * oT * rc)
        eb1w = np.ascontiguousarray(
            np.repeat((eb1s * 2.0)[:, :, None], 32, axis=2)
            .reshape(128, 2 * RPC * 32).astype(np.float16))
        in_maps.append({
            "qx": qxp, "kvx": kvp, "wqt": wqt, "wkt": wkt, "wvt": wvt,
            "wgt": wgt, "wot": wot, "eb1s": eb1s, "eb1w": eb1w, "eb2": eb2,
        })
    return in_maps


def gather(results):
    out = np.empty((1, S, Q, CQ), dtype=np.float32)
    for c in range(NCORES):
        out[0, c * RPC:(c + 1) * RPC] = results[c]["out"].astype(np.float32)
    return out


_NC_CACHE = None


def kernel_traced(q_x, kv_x, bias1, bias2, wq, wk, wv, wg, wo, trace=False):
    """Returns (full output [1,128,256,256] fp32, BassKernelResults)."""
    from concourse.bass_utils import run_bass_kernel_spmd
    global _NC_CACHE
    if _NC_CACHE is None:
        _NC_CACHE = build_nc()
    q_x, kv_x = np.asarray(q_x), np.asarray(kv_x)
    bias1, bias2 = np.asarray(bias1), np.asarray(bias2)
    wq, wk, wv, wg, wo = (np.asarray(w) for w in (wq, wk, wv, wg, wo))
    in_maps = host_prep(q_x, kv_x, bias1, bias2, wq, wk, wv, wg, wo)
    res = run_bass_kernel_spmd(_NC_CACHE, in_maps, list(range(NCORES)),
                               trace=trace)
    return gather(res.results), res


def kernel(q_x, kv_x, bias1, bias2, wq, wk, wv, wg, wo):
    """Full (unsharded) inputs in, full output out. Shards the 128 MSA
    rows across the 8 NeuronCores internally."""
    out, _ = kernel_traced(q_x, kv_x, bias1, bias2, wq, wk, wv, wg, wo)
    return out

